# revision 36
# baseline (speedup 1.0000x reference)
"""Trainium2 Bass kernel for nn_BiLSTMGenerator (BiLSTM -> LSTM -> dense).

Data-parallel over batch on 8 cores; per core B_shard = 2048 = 16 b-tiles.

v2 layout: per-tile hidden-state transposes are stacked across PSUM
partitions so each half-step needs ONE [128,256] DVE evacuation instead of
two [16,512] ones, and gate matmuls split into an h-part (stacked lhsT) and
an x/bias-part (PE is cheap, Act/DVE are the bottleneck engines).

Phase 1: fwd+bwd H=16 LSTMs fused, 2 half-step groups of 8 b-tiles.
Gate PSUM per group: [jj(8), dir(2), gate(4)x16]; elementwise 3D APs.
h^T kept stacked: partition 32*(jj%4)+16*dir+h, col 256*G+128*(jj//4)+b.
hfb scratch in DRAM stays de-interleaved [t, 32, BS] (2 scatter-stores per
step on the Pool DMA queue) so phase 2 loads it with one plain DMA.

Phase 2: mid H=64 LSTM; 4 groups of 4 b-tiles, gate PSUM [jj(4) x 256]
(two tiles per bank, 2 bufs to overlap steps), h_m^T stacked at partition
64*(j%2), col 128*(j//2). Dense tap = tiny PE matmuls (Wd @ h_m) into a
dedicated PSUM bank holding 32 steps of taps, evacuated in bulk by DVE.

Gate order (i, f, o, g). All matmul I/O bf16, PSUM fp32, states bf16.
No gpsimd tensor ops (measured ~35x slower than the cost model on HW);
gpsimd (Pool) queue is used only to issue the hfb store DMAs.
"""
import sys

sys.path.insert(0, "/opt/trn_rl_repo")

import numpy as np
import ml_dtypes

BF16NP = ml_dtypes.bfloat16

T, B, IN, H1, H2 = 216, 16384, 8, 16, 64
NCORES = 8
BS = B // NCORES  # 2048
NT = BS // 128  # 16 b-tiles


def _perm4(H):
    # torch gate order (i, f, g, o) -> (i, f, o, g)
    return np.concatenate(
        [np.arange(0, 2 * H), np.arange(3 * H, 4 * H), np.arange(2 * H, 3 * H)]
    )


def build_program(t_steps=T):
    import concourse.bass as bass
    import concourse.tile as tile
    from concourse import bacc, mybir
    from contextlib import ExitStack

    F32 = mybir.dt.float32
    BF = mybir.dt.bfloat16
    AF = mybir.ActivationFunctionType

    nc = bacc.Bacc("TRN2", target_bir_lowering=False, debug=False)

    xpad_d = nc.declare_dram_parameter("xpad", [t_steps, 9, BS], BF, isOutput=False)
    rhsh1_d = nc.declare_dram_parameter("rhs_h1", [128, 128], BF, isOutput=False)
    rhsx1_d = nc.declare_dram_parameter("rhs_x1", [18, 128], BF, isOutput=False)
    rhsfb_d = nc.declare_dram_parameter("rhs_fb", [128, 256], BF, isOutput=False)
    rhsfbF_d = nc.declare_dram_parameter("rhs_fbF", [128, 256], BF, isOutput=False)
    rhsbm_d = nc.declare_dram_parameter("rhs_bm", [1, 512], BF, isOutput=False)
    rhshm_d = nc.declare_dram_parameter("rhs_hm", [128, 256], BF, isOutput=False)
    rhsd_d = nc.declare_dram_parameter("rhs_d", [128, 1], BF, isOutput=False)
    h0p1_d = nc.declare_dram_parameter("h0p1", [128, 512], BF, isOutput=False)
    h0m_d = nc.declare_dram_parameter("h0mS", [128, 1024], BF, isOutput=False)
    c0p1_d = nc.declare_dram_parameter("c0p1", [128, 512], BF, isOutput=False)
    c0m_d = nc.declare_dram_parameter("c0m", [128, 1024], BF, isOutput=False)
    id_d = nc.declare_dram_parameter("ident", [128, 128], BF, isOutput=False)
    ones_d = nc.declare_dram_parameter("onesrow", [1, BS], BF, isOutput=False)
    out_d = nc.declare_dram_parameter("out", [128, t_steps * 16], F32, isOutput=True)
    # stacked h^T images: image k = [fwd h^k | bwd h^{T-1-k}] in HT layout
    hfb_d = nc.dram_tensor("hfb_scratch", [t_steps, 128, 512], BF)

    with tile.TileContext(nc) as tc, ExitStack() as ctx:
        const = ctx.enter_context(tc.tile_pool(name="const", bufs=1))
        state = ctx.enter_context(tc.tile_pool(name="state", bufs=1))
        work = ctx.enter_context(tc.tile_pool(name="work", bufs=2))

        # constants (rhs_h1/rhs_hm/rhs_d replicated per PE quadrant so the
        # moving operand's base partition matches the stacked stationary's)
        rhsh1 = const.tile([128, 128], BF, tag="rhsh1")
        rhsx1 = const.tile([18, 128], BF, tag="rhsx1")
        rhsfb = const.tile([128, 256], BF, tag="rhsfb")
        rhsfbF = const.tile([128, 256], BF, tag="rhsfbF")
        rhsbm = const.tile([1, 512], BF, tag="rhsbm")
        rhshm = const.tile([128, 256], BF, tag="rhshm")
        rhsd = const.tile([128, 1], BF, tag="rhsd")
        ident = const.tile([128, 128], BF, tag="ident")
        ones1 = const.tile([1, 128], BF, tag="ones1")
        nc.sync.dma_start(out=rhsh1[:, :], in_=rhsh1_d[:, :])
        nc.sync.dma_start(out=rhsx1[:, :], in_=rhsx1_d[:, :])
        nc.sync.dma_start(out=rhsfb[:, :], in_=rhsfb_d[:, :])
        nc.sync.dma_start(out=rhsfbF[:, :], in_=rhsfbF_d[:, :])
        nc.sync.dma_start(out=rhsbm[:, :], in_=rhsbm_d[:, :])
        nc.sync.dma_start(out=rhshm[:, :], in_=rhshm_d[:, :])
        nc.sync.dma_start(out=rhsd[:, :], in_=rhsd_d[:, :])
        nc.sync.dma_start(out=ident[:, :], in_=id_d[:, :])
        nc.sync.dma_start(out=ones1[:, :], in_=ones_d[0:1, 0:128])

        # persistent state
        c1 = state.tile([128, 512], BF, tag="c1")  # [G, jj, dir, h]
        c2 = state.tile([128, 1024], BF, tag="c2")  # [j, h64]
        HTs = [state.tile([128, 512], BF, tag=f"HT{p}", name=f"HT{p}") for p in range(2)]
        HMTs = [
            state.tile([128, 1024], BF, tag=f"HMT{p}", name=f"HMT{p}") for p in range(2)
        ]
        # rows 0:9 = [x_tf; ones], rows 9:18 = [x_tb; ones]
        Xfbs = [
            state.tile([18, BS], BF, tag=f"Xfb{p}", name=f"Xfb{p}") for p in range(2)
        ]
        V2s = [
            state.tile([128, 512], BF, tag=f"V2{p}", name=f"V2{p}") for p in range(2)
        ]
        out_sb = state.tile([128, t_steps * 16], F32, tag="out_sb")

        # ------------- phase 1: fwd + bwd LSTMs fused -------------
        with tc.tile_pool(name="ps1", bufs=2, space="PSUM") as ps1, tc.tile_pool(
            name="ps1t", bufs=2, space="PSUM"
        ) as ps1t:
            nc.sync.dma_start(out=HTs[1][:, :], in_=h0p1_d[:, :])
            nc.sync.dma_start(out=c1[:, :], in_=c0p1_d[:, :])
            nc.sync.dma_start(out=Xfbs[0][0:9, :], in_=xpad_d[0])
            nc.sync.dma_start(out=Xfbs[0][9:18, :], in_=xpad_d[t_steps - 1])

            for k in range(t_steps):
                tf = k
                tb = t_steps - 1 - k
                par, nxt = k % 2, (k + 1) % 2
                HTr, HTw = HTs[nxt], HTs[par]
                Xfb = Xfbs[par]
                if k + 1 < t_steps:
                    nc.sync.dma_start(out=Xfbs[nxt][0:9, :], in_=xpad_d[tf + 1])
                    nc.sync.dma_start(out=Xfbs[nxt][9:18, :], in_=xpad_d[tb - 1])
                for G in range(2):
                    psf = ps1.tile([128, 1024], F32, tag="psf")
                    for jj in range(8):
                        j = G * 8 + jj
                        po = 32 * (jj % 4)
                        co = 256 * G + 128 * (jj // 4)
                        o = psf[:, jj * 128 : (jj + 1) * 128]
                        # K=32 fused fwd+bwd h-part: rhs is block-diagonal
                        # [Whh_f^T 0; 0 Whh_b^T], replicated per quadrant
                        nc.tensor.matmul(
                            o,
                            HTr[po : po + 32, co : co + 128],
                            rhsh1[po : po + 32, :],
                            start=True,
                            stop=False,
                            tile_position=(po, 0),
                        )
                        # K=18 x+bias part, block-diag rhs: fwd x rows hit
                        # [Wih_f^T | 0], bwd x rows hit [0 | Wih_b^T]
                        nc.tensor.matmul(
                            o,
                            Xfb[0:18, j * 128 : (j + 1) * 128],
                            rhsx1[0:18, :],
                            start=False,
                            stop=True,
                        )
                    sig = work.tile([128, 768], BF, tag="sig")
                    tgt = work.tile([128, 256], BF, tag="tgt")
                    tcc = work.tile([128, 256], BF, tag="tcc")
                    t1 = work.tile([128, 256], BF, tag="t1")
                    t2 = work.tile([128, 256], BF, tag="t2")
                    hsb = work.tile([128, 256], BF, tag="hsb")
                    psv = psf[:, :].rearrange("p (jd c) -> p jd c", c=64)
                    sigv = sig[:, :].rearrange("p (jd c) -> p jd c", c=48)
                    tgv = tgt[:, :].rearrange("p (jd c) -> p jd c", c=16)
                    tccv = tcc[:, :].rearrange("p (jd c) -> p jd c", c=16)
                    t1v = t1[:, :].rearrange("p (jd c) -> p jd c", c=16)
                    t2v = t2[:, :].rearrange("p (jd c) -> p jd c", c=16)
                    hv = hsb[:, :].rearrange("p (jd c) -> p jd c", c=16)
                    cG = c1[:, G * 256 : (G + 1) * 256].rearrange(
                        "p (jd c) -> p jd c", c=16
                    )
                    nc.scalar.activation(sigv[:, :, :], psv[:, :, 0:48], AF.Sigmoid)
                    nc.scalar.activation(tgv[:, :, :], psv[:, :, 48:64], AF.Tanh)
                    nc.vector.tensor_mul(t1v[:, :, :], sigv[:, :, 16:32], cG[:, :, :])
                    nc.vector.tensor_mul(t2v[:, :, :], sigv[:, :, 0:16], tgv[:, :, :])
                    nc.vector.tensor_add(cG[:, :, :], t1v[:, :, :], t2v[:, :, :])
                    nc.scalar.activation(tccv[:, :, :], cG[:, :, :], AF.Tanh)
                    nc.vector.tensor_mul(hv[:, :, :], sigv[:, :, 32:48], tccv[:, :, :])
                    pst = ps1t.tile([128, 256], BF, tag="pst")
                    for jj in range(8):
                        nc.tensor.transpose(
                            pst[
                                32 * (jj % 4) : 32 * (jj % 4) + 32,
                                128 * (jj // 4) : 128 * (jj // 4) + 128,
                            ],
                            hsb[:, jj * 32 : (jj + 1) * 32],
                            ident[:, :],
                            tile_position=(0, 32 * (jj % 4)),
                        )
                    nc.vector.tensor_copy(
                        HTw[:, G * 256 : (G + 1) * 256], pst[:, :]
                    )
                # store the stacked image contiguously; phase 2 re-pairs
                # fwd/bwd halves across images on load
                nc.gpsimd.dma_start(out=hfb_d[k, :, :], in_=HTw[:, :])

        tc.strict_bb_all_engine_barrier()

        # ------------- phase 2: mid LSTM + dense taps -------------
        with tc.tile_pool(name="ps2", bufs=2, space="PSUM") as ps2, tc.tile_pool(
            name="ps2t", bufs=2, space="PSUM"
        ) as ps2t, tc.tile_pool(name="ps2d", bufs=2, space="PSUM") as ps2d:
            import bass_rust as _br

            def load_v2(dstile, t2):
                # per quadrant: fwd half from image t2, bwd half from image
                # T-1-t2, paired via a custom 2-entry outer stride. The BIR
                # verifier rejects negative strides, so when the bwd image
                # precedes (t2 in the second half) we load [hb; hf] order
                # instead and the consumer uses the row-flipped rhs.
                tA, tB = t2, t_steps - 1 - t2
                if tB >= tA:
                    S = (tB - tA) * 128 * 512 + 16 * 512
                    base = tA * 128 * 512
                else:
                    S = (tA - tB) * 128 * 512 - 16 * 512
                    base = tB * 128 * 512 + 16 * 512
                for q in range(4):
                    src = hfb_d[0, :, :].copy()
                    src.ap = _br.VecI64Pair([(S, 2), (512, 16), (1, 512)])
                    src.offset = base + (32 * q) * 512
                    nc.sync.dma_start(
                        out=dstile[32 * q : 32 * q + 32, :], in_=src
                    )

            nc.sync.dma_start(out=HMTs[1][:, :], in_=h0m_d[:, :])
            nc.sync.dma_start(out=c2[:, :], in_=c0m_d[:, :])
            load_v2(V2s[0], 0)

            psd = None
            for t in range(t_steps + 1):
                par, nxt = t % 2, (t + 1) % 2
                HMr = HMTs[nxt]  # h_m^{t-1}
                # taps for step t-1 from HMr
                if t >= 1:
                    ti = t - 1
                    blk, pos = ti // 32, ti % 32
                    if pos == 0:
                        psd = ps2d.tile([128, 512], F32, tag="psd")
                    for j in range(16):
                        po = 64 * (j % 2)
                        nc.tensor.matmul(
                            psd[:, pos * 16 + j : pos * 16 + j + 1],
                            HMr[po : po + 64, 128 * (j // 2) : 128 * (j // 2) + 128],
                            rhsd[po : po + 64, :],
                            start=True,
                            stop=True,
                        )
                    if pos == 31 or ti == t_steps - 1:
                        nc.vector.tensor_copy(
                            out_sb[:, blk * 512 : blk * 512 + (pos + 1) * 16],
                            psd[:, 0 : (pos + 1) * 16],
                        )
                if t == t_steps:
                    break
                if t + 1 < t_steps:
                    load_v2(V2s[nxt], t + 1)
                V2 = V2s[par]
                HMw = HMTs[par]
                rfb = rhsfbF if 2 * t > t_steps - 1 else rhsfb
                for g in range(4):
                    psm = ps2.tile([128, 1024], F32, tag="psm")
                    # bias via K=1 ones matmul, one per PSUM bank (2 tiles)
                    for b2 in range(2):
                        nc.tensor.matmul(
                            psm[:, b2 * 512 : (b2 + 1) * 512],
                            ones1[0:1, :],
                            rhsbm[:, :],
                            start=True,
                            stop=False,
                            skip_group_check=True,
                        )
                    for jj in range(4):
                        j = 4 * g + jj
                        o = psm[:, jj * 256 : (jj + 1) * 256]
                        # stacked image: tile j at partitions 32*jj, col 128*g
                        po2 = 32 * jj
                        nc.tensor.matmul(
                            o,
                            V2[po2 : po2 + 32, 128 * g : 128 * g + 128],
                            rfb[po2 : po2 + 32, :],
                            start=False,
                            stop=False,
                            tile_position=(po2, 0),
                            skip_group_check=True,
                        )
                        po = 64 * (j % 2)
                        nc.tensor.matmul(
                            o,
                            HMr[po : po + 64, 128 * (j // 2) : 128 * (j // 2) + 128],
                            rhshm[po : po + 64, :],
                            start=False,
                            stop=(jj % 2 == 1),
                            skip_group_check=True,
                        )
                    sigm = work.tile([128, 768], BF, tag="sigm")
                    tgm = work.tile([128, 256], BF, tag="tgm")
                    tcm = work.tile([128, 256], BF, tag="tcm")
                    t1m = work.tile([128, 256], BF, tag="t1m")
                    t2m = work.tile([128, 256], BF, tag="t2m")
                    hm = work.tile([128, 256], BF, tag="hm")
                    psv = psm[:, :].rearrange("p (j c) -> p j c", c=256)
                    sigmv = sigm[:, :].rearrange("p (j c) -> p j c", c=192)
                    tgmv = tgm[:, :].rearrange("p (j c) -> p j c", c=64)
                    tcmv = tcm[:, :].rearrange("p (j c) -> p j c", c=64)
                    t1mv = t1m[:, :].rearrange("p (j c) -> p j c", c=64)
                    t2mv = t2m[:, :].rearrange("p (j c) -> p j c", c=64)
                    hmv = hm[:, :].rearrange("p (j c) -> p j c", c=64)
                    cg = c2[:, g * 256 : (g + 1) * 256].rearrange(
                        "p (j c) -> p j c", c=64
                    )
                    nc.scalar.activation(sigmv[:, :, :], psv[:, :, 0:192], AF.Sigmoid)
                    nc.scalar.activation(tgmv[:, :, :], psv[:, :, 192:256], AF.Tanh)
                    nc.vector.tensor_mul(
                        t1mv[:, :, :], sigmv[:, :, 64:128], cg[:, :, :]
                    )
                    nc.vector.tensor_mul(
                        t2mv[:, :, :], sigmv[:, :, 0:64], tgmv[:, :, :]
                    )
                    nc.vector.tensor_add(cg[:, :, :], t1mv[:, :, :], t2mv[:, :, :])
                    nc.scalar.activation(tcmv[:, :, :], cg[:, :, :], AF.Tanh)
                    nc.vector.tensor_mul(
                        hmv[:, :, :], sigmv[:, :, 128:192], tcmv[:, :, :]
                    )
                    pstm = ps2t.tile([128, 256], BF, tag="pstm")
                    for q in range(2):
                        nc.tensor.transpose(
                            pstm[:, q * 128 : (q + 1) * 128],
                            hm[:, q * 128 : (q + 1) * 128],
                            ident[:, :],
                        )
                    nc.vector.tensor_copy(
                        HMw[:, g * 256 : (g + 1) * 256], pstm[:, :]
                    )

            nc.sync.dma_start(out=out_d[:, :], in_=out_sb[:, :])

    nc.finalize()
    return nc


def prepare_inputs(inputs, t_steps=T):
    """Build the per-core input maps (list of dicts) from full inputs."""
    f32 = np.float32
    x = np.asarray(inputs["x"], dtype=f32)[:t_steps]  # [T, B, 8]

    p1 = _perm4(H1)
    p2 = _perm4(H2)

    # phase-1 fused fwd+bwd rhs: block-diagonal [32, 128] h-part replicated
    # per PE quadrant, and [18, 128] x+bias part (cols = [fwd 64 | bwd 64],
    # gate order (i,f,o,g) within each half)
    Whh_f = np.asarray(inputs["Whh_f"], f32)[p1]
    Whh_b = np.asarray(inputs["Whh_b"], f32)[p1]
    Wih_f = np.asarray(inputs["Wih_f"], f32)[p1]
    Wih_b = np.asarray(inputs["Wih_b"], f32)[p1]
    b_f = (np.asarray(inputs["bih_f"], f32) + np.asarray(inputs["bhh_f"], f32))[p1]
    b_b = (np.asarray(inputs["bih_b"], f32) + np.asarray(inputs["bhh_b"], f32))[p1]
    blk = np.zeros((32, 128), f32)
    blk[0:16, 0:64] = Whh_f.T
    blk[16:32, 64:128] = Whh_b.T
    rhs_h1 = np.tile(blk, (4, 1)).astype(BF16NP)  # [128, 128]
    rhs_x1 = np.zeros((18, 128), f32)
    rhs_x1[0:8, 0:64] = Wih_f.T
    rhs_x1[8, 0:64] = b_f
    rhs_x1[9:17, 64:128] = Wih_b.T
    rhs_x1[17, 64:128] = b_b
    rhs_x1 = rhs_x1.astype(BF16NP)

    Wih_m = np.asarray(inputs["Wih_m"], f32)[p2]  # [256, 32]
    Whh_m = np.asarray(inputs["Whh_m"], f32)[p2]  # [256, 64]
    b_m = (np.asarray(inputs["bih_m"], f32) + np.asarray(inputs["bhh_m"], f32))[p2]
    Wd = np.asarray(inputs["Wd"], f32)[0]  # [64]

    fbblk = np.zeros((32, 256), f32)
    fbblk[0:16] = Wih_m[:, 0:16].T  # h_f rows
    fbblk[16:32] = Wih_m[:, 16:32].T  # h_b rows
    rhs_fb = np.tile(fbblk, (4, 1)).astype(BF16NP)  # [128, 256]
    fbblkF = np.zeros((32, 256), f32)
    fbblkF[0:16] = Wih_m[:, 16:32].T  # h_b rows (flipped-order load)
    fbblkF[16:32] = Wih_m[:, 0:16].T  # h_f rows
    rhs_fbF = np.tile(fbblkF, (4, 1)).astype(BF16NP)
    rhs_bm = np.tile(b_m, 2)[None, :].astype(BF16NP)  # [1, 512]
    rhs_hm = np.tile(Whh_m.T, (2, 1)).astype(BF16NP)  # [128, 256]
    rhs_d = np.tile(Wd.reshape(64, 1), (2, 1)).astype(BF16NP)  # [128, 1]

    ident = np.eye(128, dtype=BF16NP)
    onesrow = np.ones((1, BS), BF16NP)

    h0f = np.asarray(inputs["h0f"], f32)
    h0b = np.asarray(inputs["h0b"], f32)
    c0f = np.asarray(inputs["c0f"], f32)
    c0b = np.asarray(inputs["c0b"], f32)
    h0m = np.asarray(inputs["h0m"], f32)
    c0m = np.asarray(inputs["c0m"], f32)

    in_maps = []
    for c in range(NCORES):
        bs = c * BS
        xc = x[:, bs : bs + BS, :]  # [T, 2048, 8]
        xpad = np.ones((t_steps, 9, BS), f32)
        xpad[:, 0:8, :] = xc.transpose(0, 2, 1)

        # phase-1 stacked h0: [32*(jj%4)+16d+h, 256*G + 128*(jj//4) + b]
        h0p1 = np.zeros((128, 512), f32)
        c0p1 = np.zeros((128, 512), f32)
        for j in range(NT):
            G, jj = j // 8, j % 8
            rows = bs + j * 128
            po = 32 * (jj % 4)
            co = 256 * G + 128 * (jj // 4)
            h0p1[po : po + 16, co : co + 128] = h0f[rows : rows + 128].T
            h0p1[po + 16 : po + 32, co : co + 128] = h0b[rows : rows + 128].T
            c0p1[:, 256 * G + 32 * jj : 256 * G + 32 * jj + 16] = c0f[
                rows : rows + 128
            ]
            c0p1[:, 256 * G + 32 * jj + 16 : 256 * G + 32 * jj + 32] = c0b[
                rows : rows + 128
            ]

        # phase-2 stacked h0m: [64*(j%2)+h, 128*(j//2)+b]; c0m: [b, j*64+h]
        h0mS = np.zeros((128, 1024), f32)
        c0mS = np.zeros((128, 1024), f32)
        for j in range(NT):
            rows = bs + j * 128
            h0mS[64 * (j % 2) : 64 * (j % 2) + 64, 128 * (j // 2) : 128 * (j // 2) + 128] = h0m[rows : rows + 128].T
            c0mS[:, j * 64 : (j + 1) * 64] = c0m[rows : rows + 128]

        in_maps.append(
            {
                "xpad": xpad.astype(BF16NP),
                "rhs_h1": rhs_h1,
                "rhs_x1": rhs_x1,
                "rhs_fb": rhs_fb,
                "rhs_fbF": rhs_fbF,
                "rhs_bm": rhs_bm,
                "rhs_hm": rhs_hm,
                "rhs_d": rhs_d,
                "h0p1": h0p1.astype(BF16NP),
                "h0mS": h0mS.astype(BF16NP),
                "c0p1": c0p1.astype(BF16NP),
                "c0m": c0mS.astype(BF16NP),
                "ident": ident,
                "onesrow": onesrow,
            }
        )
    return in_maps


def unshard_output(results, bd, t_steps=T):
    outs = []
    for c in range(NCORES):
        oc = np.asarray(results[c]["out"], dtype=np.float32)  # [128, T*16]
        # col = t*16 + b-tile index
        oc = oc.reshape(128, t_steps, NT).transpose(2, 0, 1).reshape(BS, t_steps)
        outs.append(oc)
    full = np.concatenate(outs, axis=0)  # [B, T]
    full += bd  # dense bias applied host-side
    return full


_CACHED = {}


def kernel(**inputs):
    from concourse.bass_utils import run_bass_kernel_spmd

    t_steps = T
    if "prog" not in _CACHED:
        _CACHED["prog"] = build_program(t_steps)
    nc = _CACHED["prog"]
    in_maps = prepare_inputs(inputs, t_steps)
    res = run_bass_kernel_spmd(nc, in_maps, list(range(NCORES)))
    bd = float(np.asarray(inputs["bd"], np.float32)[0])
    return unshard_output(res.results, bd, t_steps)


if __name__ == "__main__":
    import reference

    inputs = reference.setup_inputs()
    out = kernel(**{k: np.asarray(v) for k, v in inputs.items()})
    print("kernel out", out.shape, out.dtype)


# revision 49
# speedup vs baseline: 1.0323x; 1.0323x over previous
"""Trainium2 Bass kernel for nn_BiLSTMGenerator (BiLSTM -> LSTM -> dense).

Data-parallel over batch on 8 cores; per core B_shard = 2048 = 16 b-tiles.

Phase 1: fwd+bwd H=16 LSTMs fused — per b-tile the gate PSUM holds
[fwd 64 | bwd 64] so one sigmoid/tanh/mul instruction (4D AP, dims
p/tile/dir/gate) processes both directions. 2 groups of 8 b-tiles per step.
x is loaded as bare 8 rows (ones row is a persistent memset).

Phase 2: mid H=64 LSTM + fused dense tap, 4 groups of 4 b-tiles,
lhsT V2 rows 0:32 hfbT (DMA), 32:64 ones+zeros, 64:128 hmT (PE-transpose
evac). rhs [128, 257]; col 256 taps Wd @ h_m + bd.

Gate order (i, f, o, g). All matmul I/O bf16, PSUM fp32, states bf16.
No gpsimd tensor ops (measured ~35x slower than the cost model on HW);
the hfb stores are Pool-issued DMAs to keep the SP queue short.
"""
import sys

sys.path.insert(0, "/opt/trn_rl_repo")

import numpy as np
import ml_dtypes

BF16NP = ml_dtypes.bfloat16

T, B, IN, H1, H2 = 216, 16384, 8, 16, 64
NCORES = 8
BS = B // NCORES  # 2048
NT = BS // 128  # 16 b-tiles


def _perm4(H):
    # torch gate order (i, f, g, o) -> (i, f, o, g)
    return np.concatenate(
        [np.arange(0, 2 * H), np.arange(3 * H, 4 * H), np.arange(2 * H, 3 * H)]
    )


def build_program(t_steps=T):
    import concourse.bass as bass
    import concourse.tile as tile
    from concourse import bacc, mybir
    from contextlib import ExitStack

    F32 = mybir.dt.float32
    BF = mybir.dt.bfloat16
    AF = mybir.ActivationFunctionType

    nc = bacc.Bacc("TRN2", target_bir_lowering=False, debug=False)

    xpad_d = nc.declare_dram_parameter("xpad", [t_steps, 8, BS], BF, isOutput=False)
    rhsf_d = nc.declare_dram_parameter("rhs_f", [25, 64], BF, isOutput=False)
    rhsb_d = nc.declare_dram_parameter("rhs_b", [25, 64], BF, isOutput=False)
    rhsm_d = nc.declare_dram_parameter("rhs_m", [128, 257], BF, isOutput=False)
    rhsd_d = nc.declare_dram_parameter("rhs_d", [64, 1], BF, isOutput=False)
    h0f_d = nc.declare_dram_parameter("h0fT", [16, BS], BF, isOutput=False)
    h0b_d = nc.declare_dram_parameter("h0bT", [16, BS], BF, isOutput=False)
    h0m_d = nc.declare_dram_parameter("h0mT", [64, BS], BF, isOutput=False)
    c0_d = nc.declare_dram_parameter("c0", [128, 96 * NT], BF, isOutput=False)
    id_d = nc.declare_dram_parameter("ident", [128, 128], BF, isOutput=False)
    ones_d = nc.declare_dram_parameter("onespad", [32, BS], BF, isOutput=False)
    out_d = nc.declare_dram_parameter("out", [128, t_steps * NT], F32, isOutput=True)
    hfb_d = nc.dram_tensor("hfb_scratch", [t_steps, 32, BS], BF)

    with tile.TileContext(nc) as tc, ExitStack() as ctx:
        const = ctx.enter_context(tc.tile_pool(name="const", bufs=1))
        state = ctx.enter_context(tc.tile_pool(name="state", bufs=1))
        work = ctx.enter_context(tc.tile_pool(name="work", bufs=2))

        # constants
        rhsf = const.tile([25, 64], BF, tag="rhsf")
        rhsb = const.tile([25, 64], BF, tag="rhsb")
        rhsm = const.tile([128, 257], BF, tag="rhsm")
        rhsd = const.tile([128, 1], BF, tag="rhsd")
        ident = const.tile([128, 128], BF, tag="ident")
        nc.sync.dma_start(out=rhsf[:, :], in_=rhsf_d[:, :])
        nc.sync.dma_start(out=rhsb[:, :], in_=rhsb_d[:, :])
        nc.sync.dma_start(out=rhsm[:, :], in_=rhsm_d[:, :])
        nc.sync.dma_start(out=rhsd[64:128, :], in_=rhsd_d[:, :])
        nc.sync.dma_start(out=ident[:, :], in_=id_d[:, :])

        # persistent state
        # c_fb interleaved per b-tile: [t(16), dir(2), c(16)] -> 512 cols
        # c_m: [t(16), c(64)] -> 1024 cols
        c_all = state.tile([128, 96 * NT], BF, tag="c_all")
        Fbs = [
            state.tile([32, BS], BF, tag=f"Fb{i}", name=f"Fb{i}") for i in range(2)
        ]
        Abs_ = [
            state.tile([32, BS], BF, tag=f"Ab{i}", name=f"Ab{i}") for i in range(2)
        ]
        V2s = [
            state.tile([128, BS], BF, tag=f"V2{i}", name=f"V2{i}") for i in range(2)
        ]
        out_sb = state.tile([128, t_steps * NT], F32, tag="out_sb")
        nc.sync.dma_start(out=c_all[:, :], in_=c0_d[:, :])

        C_FB = slice(0, 32 * NT)
        C_M = slice(32 * NT, 96 * NT)

        # ------------- phase 1: fwd + bwd LSTMs fused -------------
        with tc.tile_pool(name="ps1", bufs=4, space="PSUM") as ps1, tc.tile_pool(
            name="ps1t", bufs=2, space="PSUM"
        ) as ps1t:
            for i in range(2):
                nc.sync.dma_start(out=Fbs[i][24:25, :], in_=ones_d[0:1, :])
                nc.sync.dma_start(out=Abs_[i][24:25, :], in_=ones_d[0:1, :])

            nc.sync.dma_start(out=Fbs[0][0:16, :], in_=h0f_d[:, :])
            nc.sync.dma_start(out=Fbs[0][16:24, :], in_=xpad_d[0])
            nc.sync.dma_start(out=Abs_[0][0:16, :], in_=h0b_d[:, :])
            nc.sync.dma_start(out=Abs_[0][16:24, :], in_=xpad_d[t_steps - 1])

            cv_all = c_all[:, C_FB].rearrange("p (t h c) -> p t h c", h=2, c=16)

            for k in range(t_steps):
                tf = k
                tb = t_steps - 1 - k
                cur, nxt = k % 2, (k + 1) % 2
                Fb, Fbn = Fbs[cur], Fbs[nxt]
                Ab, Abn = Abs_[cur], Abs_[nxt]
                if k + 1 < t_steps:
                    nc.sync.dma_start(out=Fbn[16:24, :], in_=xpad_d[tf + 1])
                    nc.sync.dma_start(out=Abn[16:24, :], in_=xpad_d[tb - 1])
                for g in range(4):
                    t0 = 4 * g
                    sig = work.tile([128, 384], BF, tag=f"sig{g}", name=f"sig{g}")
                    tg = work.tile([128, 128], BF, tag=f"tg{g}", name=f"tg{g}")
                    tcc = work.tile([128, 128], BF, tag=f"tc{g}", name=f"tc{g}")
                    h = work.tile([128, 128], BF, tag=f"h{g}", name=f"h{g}")
                    t1 = work.tile([128, 128], BF, tag=f"t1{g}", name=f"t1{g}")
                    t2 = work.tile([128, 128], BF, tag=f"t2{g}", name=f"t2{g}")
                    psf = ps1.tile([128, 512], F32, tag="psf")
                    for j in range(4):
                        jt = t0 + j
                        nc.tensor.matmul(
                            psf[:, j * 128 : j * 128 + 64],
                            Fb[0:25, jt * 128 : (jt + 1) * 128],
                            rhsf[:, :],
                            start=True,
                            stop=True,
                        )
                        nc.tensor.matmul(
                            psf[:, j * 128 + 64 : (j + 1) * 128],
                            Ab[0:25, jt * 128 : (jt + 1) * 128],
                            rhsb[:, :],
                            start=True,
                            stop=True,
                        )
                    psv = psf[:, :].rearrange("p (t h c) -> p t h c", h=2, c=64)
                    sigv = sig[:, :].rearrange("p (t h c) -> p t h c", h=2, c=48)
                    tgv = tg[:, :].rearrange("p (t h c) -> p t h c", h=2, c=16)
                    t1v = t1[:, :].rearrange("p (t h c) -> p t h c", h=2, c=16)
                    t2v = t2[:, :].rearrange("p (t h c) -> p t h c", h=2, c=16)
                    tccv = tcc[:, :].rearrange("p (t h c) -> p t h c", h=2, c=16)
                    hv = h[:, :].rearrange("p (t h c) -> p t h c", h=2, c=16)
                    cv = cv_all[:, t0 : t0 + 4, :, :]
                    nc.scalar.activation(sigv[:, :, :, :], psv[:, :, :, 0:48], AF.Sigmoid)
                    nc.scalar.activation(tgv[:, :, :, :], psv[:, :, :, 48:64], AF.Tanh)
                    nc.vector.tensor_mul(
                        t1v[:, :, :, :], sigv[:, :, :, 16:32], cv[:, :, :, :]
                    )
                    nc.vector.tensor_mul(
                        t2v[:, :, :, :], sigv[:, :, :, 0:16], tgv[:, :, :, :]
                    )
                    nc.vector.tensor_add(
                        cv[:, :, :, :], t1v[:, :, :, :], t2v[:, :, :, :]
                    )
                    nc.scalar.activation(tccv[:, :, :, :], cv[:, :, :, :], AF.Tanh)
                    nc.vector.tensor_mul(
                        hv[:, :, :, :], sigv[:, :, :, 32:48], tccv[:, :, :, :]
                    )
                    pst = ps1t.tile([16, 1024], BF, tag="pst")
                    for j in range(4):
                        nc.tensor.transpose(
                            pst[0:16, j * 128 : (j + 1) * 128],
                            h[:, j * 32 : j * 32 + 16],
                            ident[:, :],
                        )
                        nc.tensor.transpose(
                            pst[0:16, 512 + j * 128 : 512 + (j + 1) * 128],
                            h[:, j * 32 + 16 : (j + 1) * 32],
                            ident[:, :],
                        )
                    nc.vector.tensor_copy(
                        Fbn[0:16, g * 512 : (g + 1) * 512], pst[0:16, 0:512]
                    )
                    nc.vector.tensor_copy(
                        Abn[0:16, g * 512 : (g + 1) * 512], pst[0:16, 512:1024]
                    )
                nc.gpsimd.dma_start(out=hfb_d[tf, 0:16, :], in_=Fbn[0:16, :])
                nc.gpsimd.dma_start(out=hfb_d[tb, 16:32, :], in_=Abn[0:16, :])

        tc.strict_bb_all_engine_barrier()

        # ------------- phase 2: mid LSTM + fused dense -------------
        with tc.tile_pool(name="ps2", bufs=1, space="PSUM") as ps2, tc.tile_pool(
            name="ps2t", bufs=2, space="PSUM"
        ) as ps2t:
            nc.sync.dma_start(out=V2s[0][0:32, :], in_=hfb_d[0, :, :])
            nc.sync.dma_start(out=V2s[0][32:64, :], in_=ones_d[:, :])
            nc.sync.dma_start(out=V2s[1][32:64, :], in_=ones_d[:, :])
            nc.sync.dma_start(out=V2s[0][64:128, :], in_=h0m_d[:, :])
            for t in range(t_steps):
                cur, nxt = t % 2, (t + 1) % 2
                V2, V2n = V2s[cur], V2s[nxt]
                if t < t_steps - 1:
                    nc.sync.dma_start(out=V2n[0:32, :], in_=hfb_d[t + 1, :, :])
                sigm = work.tile([128, 192 * NT], BF, tag="sigm")
                tgm = work.tile([128, 64 * NT], BF, tag="tgm")
                tcm = work.tile([128, 64 * NT], BF, tag="tcm")
                hm = work.tile([128, 64 * NT], BF, tag="hm")
                t1m = work.tile([128, 64 * NT], BF, tag="t1m")
                t2m = work.tile([128, 64 * NT], BF, tag="t2m")
                sigmv = sigm[:, :].rearrange("p (t c) -> p t c", c=192)
                tgmv = tgm[:, :].rearrange("p (t c) -> p t c", c=64)
                cmv = c_all[:, C_M].rearrange("p (t c) -> p t c", c=64)
                t1mv = t1m[:, :].rearrange("p (t c) -> p t c", c=64)
                t2mv = t2m[:, :].rearrange("p (t c) -> p t c", c=64)
                tcmv = tcm[:, :].rearrange("p (t c) -> p t c", c=64)
                hmv = hm[:, :].rearrange("p (t c) -> p t c", c=64)
                for g in range(4):
                    sl = slice(4 * g, 4 * (g + 1))
                    psm = ps2.tile([128, 2048], F32, tag="psm")
                    for k2 in range(4):
                        jt = 4 * g + k2
                        nc.tensor.matmul(
                            psm[:, k2 * 512 : k2 * 512 + 257],
                            V2[0:128, jt * 128 : (jt + 1) * 128],
                            rhsm[:, :],
                            start=True,
                            stop=True,
                        )
                    psv = psm[:, :].rearrange("p (t c) -> p t c", c=512)
                    nc.scalar.activation(sigmv[:, sl, :], psv[:, :, 0:192], AF.Sigmoid)
                    nc.scalar.activation(tgmv[:, sl, :], psv[:, :, 192:256], AF.Tanh)
                    if t >= 1:
                        nc.scalar.copy(
                            out_sb[
                                :, (t - 1) * 16 + 4 * g : (t - 1) * 16 + 4 * g + 4
                            ].rearrange("p (a b) -> p a b", b=1),
                            psv[:, :, 256:257],
                        )
                    nc.vector.tensor_mul(
                        t1mv[:, sl, :], sigmv[:, sl, 64:128], cmv[:, sl, :]
                    )
                    nc.vector.tensor_mul(
                        t2mv[:, sl, :], sigmv[:, sl, 0:64], tgmv[:, sl, :]
                    )
                    nc.vector.tensor_add(cmv[:, sl, :], t1mv[:, sl, :], t2mv[:, sl, :])
                    nc.scalar.activation(tcmv[:, sl, :], cmv[:, sl, :], AF.Tanh)
                    nc.vector.tensor_mul(
                        hmv[:, sl, :], sigmv[:, sl, 128:192], tcmv[:, sl, :]
                    )
                    pstm = ps2t.tile([64, 512], BF, tag="pstm")
                    for j in range(4):
                        jt = 4 * g + j
                        nc.tensor.transpose(
                            pstm[0:64, j * 128 : (j + 1) * 128],
                            hm[:, jt * 64 : (jt + 1) * 64],
                            ident[:, :],
                        )
                    nc.vector.tensor_copy(
                        V2n[64:128, g * 512 : (g + 1) * 512], pstm[0:64, :]
                    )

            # final dense tap: out[T-1] = Wd @ h_m[T-1] (+bd host-side)
            psd = ps2.tile([128, 2048], F32, tag="psm")
            Vlast = V2s[t_steps % 2]
            for j in range(NT):
                nc.tensor.matmul(
                    psd[:, j : j + 1],
                    Vlast[64:128, j * 128 : (j + 1) * 128],
                    rhsd[64:128, :],
                    start=True,
                    stop=True,
                )
            nc.vector.tensor_copy(
                out_sb[:, (t_steps - 1) * 16 : t_steps * 16], psd[:, 0:16]
            )
            nc.sync.dma_start(out=out_d[:, :], in_=out_sb[:, :])

    nc.finalize()
    return nc


def prepare_inputs(inputs, t_steps=T):
    """Build the per-core input maps (list of dicts) from full inputs."""
    f32 = np.float32
    x = np.asarray(inputs["x"], dtype=f32)[:t_steps]  # [T, B, 8]

    p1 = _perm4(H1)
    p2 = _perm4(H2)

    def rhs_small(Wih, Whh, bih, bhh):
        # rows 0:16 Whh.T ; 16:24 Wih.T ; 24 bias   (cols = gates (i,f,o,g))
        Wih = np.asarray(Wih, f32)[p1]
        Whh = np.asarray(Whh, f32)[p1]
        b = (np.asarray(bih, f32) + np.asarray(bhh, f32))[p1]
        out = np.zeros((25, 4 * H1), f32)
        out[0:16] = Whh.T
        out[16:24] = Wih.T
        out[24] = b
        return out.astype(BF16NP)

    rhs_f = rhs_small(
        inputs["Wih_f"], inputs["Whh_f"], inputs["bih_f"], inputs["bhh_f"]
    )
    rhs_b = rhs_small(
        inputs["Wih_b"], inputs["Whh_b"], inputs["bih_b"], inputs["bhh_b"]
    )

    Wih_m = np.asarray(inputs["Wih_m"], f32)[p2]  # [256, 32]
    Whh_m = np.asarray(inputs["Whh_m"], f32)[p2]  # [256, 64]
    b_m = (np.asarray(inputs["bih_m"], f32) + np.asarray(inputs["bhh_m"], f32))[p2]
    Wd = np.asarray(inputs["Wd"], f32)[0]  # [64]
    bd = np.asarray(inputs["bd"], f32)[0]
    rhs_m = np.zeros((128, 257), f32)
    rhs_m[0:16, 0:256] = Wih_m[:, 0:16].T  # h_f part (V2 rows 0:16)
    rhs_m[16:32, 0:256] = Wih_m[:, 16:32].T  # h_b part (V2 rows 16:32)
    rhs_m[32, 0:256] = b_m  # ones row (V2 row 32) -> bias
    rhs_m[64:128, 0:256] = Whh_m.T  # h_m part (V2 rows 64:128)
    rhs_m[32, 256] = bd
    rhs_m[64:128, 256] = Wd
    rhs_m = rhs_m.astype(BF16NP)

    rhs_d = Wd.reshape(64, 1).astype(BF16NP)  # bd added host-side for last col

    ident = np.eye(128, dtype=BF16NP)
    onespad = np.zeros((32, BS), BF16NP)
    onespad[0, :] = 1

    in_maps = []
    for c in range(NCORES):
        bs, be = c * BS, (c + 1) * BS
        xc = x[:, bs:be, :]  # [T, 2048, 8]
        xpad = np.ascontiguousarray(xc.transpose(0, 2, 1)).astype(BF16NP)

        def bm(a, H):  # [BS, H] -> batch-major [128, NT*H]
            return (
                np.asarray(a, f32)[bs:be]
                .reshape(NT, 128, H)
                .transpose(1, 0, 2)
                .reshape(128, NT * H)
            )

        c0 = np.zeros((128, 96 * NT), f32)
        # c_fb interleaved [tile, dir, 16]: dir0=fwd, dir1=bwd
        cfb = np.stack([bm(inputs["c0f"], H1), bm(inputs["c0b"], H1)], axis=2)
        # bm gives [128, NT*16]; reshape to [128, NT, 16] then interleave
        cf = bm(inputs["c0f"], H1).reshape(128, NT, 16)
        cb = bm(inputs["c0b"], H1).reshape(128, NT, 16)
        c0[:, 0 : 32 * NT] = np.stack([cf, cb], axis=2).reshape(128, 32 * NT)
        c0[:, 32 * NT :] = bm(inputs["c0m"], H2)

        in_maps.append(
            {
                "xpad": xpad,
                "rhs_f": rhs_f,
                "rhs_b": rhs_b,
                "rhs_m": rhs_m,
                "rhs_d": rhs_d,
                "h0fT": np.asarray(inputs["h0f"], f32)[bs:be].T.astype(BF16NP),
                "h0bT": np.asarray(inputs["h0b"], f32)[bs:be].T.astype(BF16NP),
                "h0mT": np.asarray(inputs["h0m"], f32)[bs:be].T.astype(BF16NP),
                "c0": c0.astype(BF16NP),
                "ident": ident,
                "onespad": onespad,
            }
        )
    return in_maps


def unshard_output(results, bd, t_steps=T):
    outs = []
    for c in range(NCORES):
        oc = np.asarray(results[c]["out"], dtype=np.float32)  # [128, T*NT]
        # col = t*NT + b-tile index
        oc = oc.reshape(128, t_steps, NT).transpose(2, 0, 1).reshape(BS, t_steps)
        outs.append(oc)
    full = np.concatenate(outs, axis=0)  # [B, T]
    full[:, t_steps - 1] += bd  # last step's dense bias is added host-side
    return full


_CACHED = {}


def kernel(**inputs):
    from concourse.bass_utils import run_bass_kernel_spmd

    t_steps = T
    if "prog" not in _CACHED:
        _CACHED["prog"] = build_program(t_steps)
    nc = _CACHED["prog"]
    in_maps = prepare_inputs(inputs, t_steps)
    res = run_bass_kernel_spmd(nc, in_maps, list(range(NCORES)))
    bd = float(np.asarray(inputs["bd"], np.float32)[0])
    return unshard_output(res.results, bd, t_steps)


if __name__ == "__main__":
    import reference

    inputs = reference.setup_inputs()
    out = kernel(**{k: np.asarray(v) for k, v in inputs.items()})
    print("kernel out", out.shape, out.dtype)



# revision 55
# speedup vs baseline: 1.0661x; 1.0327x over previous
"""Trainium2 Bass kernel for nn_BiLSTMGenerator (BiLSTM -> LSTM -> dense). v5

Data-parallel over batch on 8 cores; per core B_shard = 2048 = 16 b-tiles.

HW constraints honored (micro-benchmarked on this device):
 - no PSUM accumulation groups (every matmul is start+stop);
 - consecutive matmuls in the same PE row-tiling mode never change row
   position (even-parity tiles batch at row 0, a 32-mode dummy matmul acts
   as a mode breaker, then odd-parity tiles at row 64);
 - col-position changes (transposes) are unrestricted.

Phase 1: fwd+bwd H=16 LSTMs. Per tile ONE K=50 matmul: stationary is
[h_f(16); h_b(16); x_f+1(9); x_b+1(9)] stacked in HT2 at row 64*(j%2),
col 128*(j//2); moving rhs is block-diagonal [50, 128] replicated at rows
0/64. Gates come out [128b, fwd 64 | bwd 64] per tile. Transposes write
h^T back at col-positions; 2 DVE copies per 8-tile group.

Phase 2: baseline-style V2 [128, 2048] (rows 0:32 hfbT de-interleaved by
the loads, 32 ones, 64:128 hmT) with ONE K=128 matmul per tile, 256-wide
gates (no tap column) so two tiles share a PSUM bank and psm double-buffers
across steps. Dense taps are batched K=64 matmuls at (64,0) into a psd bank
holding 32 steps, bulk-evacuated by DVE.

Gate order (i, f, o, g). All matmul I/O bf16, PSUM fp32, states bf16.
"""
import sys

sys.path.insert(0, "/opt/trn_rl_repo")

import numpy as np
import ml_dtypes

BF16NP = ml_dtypes.bfloat16

T, B, IN, H1, H2 = 216, 16384, 8, 16, 64
NCORES = 8
BS = B // NCORES  # 2048
NT = BS // 128  # 16 b-tiles


def _perm4(H):
    # torch gate order (i, f, g, o) -> (i, f, o, g)
    return np.concatenate(
        [np.arange(0, 2 * H), np.arange(3 * H, 4 * H), np.arange(2 * H, 3 * H)]
    )


def build_program(t_steps=T):
    import concourse.bass as bass
    import concourse.tile as tile
    from concourse import bacc, mybir
    from contextlib import ExitStack

    F32 = mybir.dt.float32
    BF = mybir.dt.bfloat16
    AF = mybir.ActivationFunctionType

    nc = bacc.Bacc("TRN2", target_bir_lowering=False, debug=False)

    xpad_d = nc.declare_dram_parameter("xpad", [t_steps, 9, BS], BF, isOutput=False)
    rhs50_d = nc.declare_dram_parameter("rhs50", [128, 128], BF, isOutput=False)
    rhsm_d = nc.declare_dram_parameter("rhs_m", [128, 256], BF, isOutput=False)
    rhsd_d = nc.declare_dram_parameter("rhs_d", [64, 1], BF, isOutput=False)
    h0p1_d = nc.declare_dram_parameter("h0p1", [128, 1024], BF, isOutput=False)
    h0m_d = nc.declare_dram_parameter("h0mT", [64, BS], BF, isOutput=False)
    c0p1_d = nc.declare_dram_parameter("c0p1", [128, 512], BF, isOutput=False)
    c0m_d = nc.declare_dram_parameter("c0m", [128, 1024], BF, isOutput=False)
    id_d = nc.declare_dram_parameter("ident", [128, 128], BF, isOutput=False)
    ones_d = nc.declare_dram_parameter("onesrow", [1, BS], BF, isOutput=False)
    out_d = nc.declare_dram_parameter("out", [128, t_steps * 16], F32, isOutput=True)
    # per step: [class (even/odd tiles), 32 (d,h), 1024 (cb, b)]
    hfb_d = nc.dram_tensor("hfb_scratch", [t_steps, 2, 32, 1024], BF)

    with tile.TileContext(nc) as tc, ExitStack() as ctx:
        const = ctx.enter_context(tc.tile_pool(name="const", bufs=1))
        state = ctx.enter_context(tc.tile_pool(name="state", bufs=1))
        work = ctx.enter_context(tc.tile_pool(name="work", bufs=2))

        rhs50 = const.tile([128, 128], BF, tag="rhs50")
        rhsm = const.tile([128, 256], BF, tag="rhsm")
        rhsd = const.tile([128, 1], BF, tag="rhsd")
        ident = const.tile([128, 128], BF, tag="ident")
        nc.sync.dma_start(out=rhs50[:, :], in_=rhs50_d[:, :])
        nc.sync.dma_start(out=rhsm[:, :], in_=rhsm_d[:, :])
        nc.sync.dma_start(out=rhsd[64:128, :], in_=rhsd_d[:, :])
        nc.sync.dma_start(out=ident[:, :], in_=id_d[:, :])

        # persistent state
        c1 = state.tile([128, 512], BF, tag="c1")  # [t16, d2, h16]
        c2 = state.tile([128, 1024], BF, tag="c2")  # [t16, h64]
        HT2s = [
            state.tile([128, 1024], BF, tag=f"HT2{p}", name=f"HT2{p}")
            for p in range(2)
        ]
        V2s = [
            state.tile([128, BS], BF, tag=f"V2{p}", name=f"V2{p}") for p in range(2)
        ]
        out_sb = state.tile([128, t_steps * 16], F32, tag="out_sb")

        ROWS_H = slice(0, 32)  # within a parity class: hf 0:16, hb 16:32
        # x rows within class: xf 32:41, xb 41:50

        def xsrc(t2, par_class):
            # xpad[t2] view [9, 8 blocks stride 256, 128] picking even/odd tiles
            v = xpad_d[t2].rearrange("r (blk b) -> r blk b", b=128)
            return v[:, par_class::2, :]

        # ------------- phase 1: fwd + bwd LSTMs fused -------------
        with tc.tile_pool(name="ps1", bufs=3, space="PSUM") as ps1, tc.tile_pool(
            name="ps1t", bufs=2, space="PSUM"
        ) as ps1t:
            nc.sync.dma_start(out=HT2s[1][:, :], in_=h0p1_d[:, :])
            nc.sync.dma_start(out=c1[:, :], in_=c0p1_d[:, :])
            for pc in range(2):
                nc.sync.dma_start(
                    out=HT2s[1][64 * pc + 32 : 64 * pc + 41, :].rearrange(
                        "r (blk b) -> r blk b", b=128
                    ),
                    in_=xsrc(0, pc),
                )
                nc.sync.dma_start(
                    out=HT2s[1][64 * pc + 41 : 64 * pc + 50, :].rearrange(
                        "r (blk b) -> r blk b", b=128
                    ),
                    in_=xsrc(t_steps - 1, pc),
                )

            for k in range(t_steps):
                tf = k
                tb = t_steps - 1 - k
                par, nxt = k % 2, (k + 1) % 2
                HTr, HTw = HT2s[nxt], HT2s[par]
                if k + 1 < t_steps:
                    for pc in range(2):
                        eng = nc.sync if pc == 0 else nc.gpsimd
                        eng.dma_start(
                            out=HTw[64 * pc + 32 : 64 * pc + 41, :].rearrange(
                                "r (blk b) -> r blk b", b=128
                            ),
                            in_=xsrc(tf + 1, pc),
                        )
                        eng.dma_start(
                            out=HTw[64 * pc + 41 : 64 * pc + 50, :].rearrange(
                                "r (blk b) -> r blk b", b=128
                            ),
                            in_=xsrc(tb - 1, pc),
                        )
                # gate matmuls: even tiles (row 0), mode-break, odd (row 64)
                psfs = [
                    ps1.tile([128, 1024], F32, tag="psf", name=f"psf{i}")
                    for i in range(2)
                ]
                for phase_par in range(2):
                    if phase_par == 1:
                        # 32-mode dummy: overwritten by tile 1's real mm
                        nc.tensor.matmul(
                            psfs[0][:, 128:129],
                            HTr[0:1, 0:128],
                            rhs50[0:1, 0:1],
                            start=True,
                            stop=True,
                        )
                    for j in range(phase_par, NT, 2):
                        G, jj = j // 8, j % 8
                        po = 64 * (j % 2)
                        cb = j // 2
                        nc.tensor.matmul(
                            psfs[G][:, jj * 128 : (jj + 1) * 128],
                            HTr[po : po + 50, cb * 128 : (cb + 1) * 128],
                            rhs50[po : po + 50, :],
                            start=True,
                            stop=True,
                            tile_position=(po, 0),
                        )
                for G in range(2):
                    psf = psfs[G]
                    sig = work.tile([128, 768], BF, tag=f"sig{G}")
                    tgt = work.tile([128, 256], BF, tag=f"tgt{G}")
                    tcc = work.tile([128, 256], BF, tag=f"tcc{G}")
                    t1 = work.tile([128, 256], BF, tag=f"t1{G}")
                    t2 = work.tile([128, 256], BF, tag=f"t2{G}")
                    hsb = work.tile([128, 256], BF, tag=f"hsb{G}")
                    psv = psf[:, :].rearrange("p (t d c) -> p t d c", d=2, c=64)
                    sigv = sig[:, :].rearrange("p (t d c) -> p t d c", d=2, c=48)
                    tgv = tgt[:, :].rearrange("p (t d c) -> p t d c", d=2, c=16)
                    tccv = tcc[:, :].rearrange("p (t d c) -> p t d c", d=2, c=16)
                    t1v = t1[:, :].rearrange("p (t d c) -> p t d c", d=2, c=16)
                    t2v = t2[:, :].rearrange("p (t d c) -> p t d c", d=2, c=16)
                    hv = hsb[:, :].rearrange("p (t d c) -> p t d c", d=2, c=16)
                    cG = c1[:, G * 256 : (G + 1) * 256].rearrange(
                        "p (t d c) -> p t d c", d=2, c=16
                    )
                    nc.scalar.activation(
                        sigv[:, :, :, :], psv[:, :, :, 0:48], AF.Sigmoid
                    )
                    nc.scalar.activation(tgv[:, :, :, :], psv[:, :, :, 48:64], AF.Tanh)
                    nc.vector.tensor_mul(
                        t1v[:, :, :, :], sigv[:, :, :, 16:32], cG[:, :, :, :]
                    )
                    nc.vector.tensor_mul(
                        t2v[:, :, :, :], sigv[:, :, :, 0:16], tgv[:, :, :, :]
                    )
                    nc.vector.tensor_add(
                        cG[:, :, :, :], t1v[:, :, :, :], t2v[:, :, :, :]
                    )
                    nc.scalar.activation(tccv[:, :, :, :], cG[:, :, :, :], AF.Tanh)
                    nc.vector.tensor_mul(
                        hv[:, :, :, :], sigv[:, :, :, 32:48], tccv[:, :, :, :]
                    )
                    # transposes: tile jj -> pst rows 64*(jj%2), col 128*(jj//2)
                    pst = ps1t.tile([128, 512], BF, tag="pst")
                    for jj in range(8):
                        ppo = 64 * (jj % 2)
                        nc.tensor.transpose(
                            pst[ppo : ppo + 32, 128 * (jj // 2) : 128 * (jj // 2) + 128],
                            hsb[:, jj * 32 : (jj + 1) * 32],
                            ident[:, :],
                            tile_position=(0, ppo),
                        )
                    # copies into HT2 h rows; G tiles 8G..8G+7 -> cb 4G..4G+3
                    nc.vector.tensor_copy(
                        HTw[0:32, G * 512 : (G + 1) * 512], pst[0:32, :]
                    )
                    nc.vector.tensor_copy(
                        HTw[64:96, G * 512 : (G + 1) * 512], pst[64:96, :]
                    )
                # stores: class rows {0:32} and {64:96}
                nc.gpsimd.dma_start(out=hfb_d[k, 0, :, :], in_=HTw[0:32, :])
                nc.gpsimd.dma_start(out=hfb_d[k, 1, :, :], in_=HTw[64:96, :])

        tc.strict_bb_all_engine_barrier()

        # ------------- phase 2: mid LSTM + dense taps -------------
        with tc.tile_pool(name="ps2", bufs=2, space="PSUM") as ps2, tc.tile_pool(
            name="ps2t", bufs=2, space="PSUM"
        ) as ps2t, tc.tile_pool(name="ps2d", bufs=2, space="PSUM") as ps2d:

            def load_v2(dst, t2):
                # de-interleave: tile j -> V2 col j*128; class pc=j%2, cb=j//2
                tA, tB = t2, t_steps - 1 - t2
                for pc in range(2):
                    eng = nc.sync if pc == 0 else nc.gpsimd
                    dv = dst[0:16, :].rearrange("p (blk b) -> p blk b", b=128)[
                        :, pc::2, :
                    ]
                    eng.dma_start(
                        out=dv,
                        in_=hfb_d[tA, pc, 0:16, :].rearrange(
                            "p (blk b) -> p blk b", b=128
                        ),
                    )
                    dv2 = dst[16:32, :].rearrange("p (blk b) -> p blk b", b=128)[
                        :, pc::2, :
                    ]
                    eng.dma_start(
                        out=dv2,
                        in_=hfb_d[tB, pc, 16:32, :].rearrange(
                            "p (blk b) -> p blk b", b=128
                        ),
                    )

            nc.vector.memset(V2s[0][32:64, :], 0.0)
            nc.vector.memset(V2s[1][32:64, :], 0.0)
            nc.sync.dma_start(out=V2s[0][32:33, :], in_=ones_d[:, :])
            nc.sync.dma_start(out=V2s[1][32:33, :], in_=ones_d[:, :])
            nc.sync.dma_start(out=V2s[0][64:128, :], in_=h0m_d[:, :])
            nc.sync.dma_start(out=c2[:, :], in_=c0m_d[:, :])
            load_v2(V2s[0], 0)

            psd = None
            for t in range(t_steps + 1):
                par, nxt = t % 2, (t + 1) % 2
                V2, V2n = V2s[par], V2s[nxt]
                # taps for step t-1 from V2[64:128] (h_m^{t-1})
                if t >= 1:
                    ti = t - 1
                    blk, pos = ti // 32, ti % 32
                    if pos == 0:
                        psd = ps2d.tile([128, 512], F32, tag="psd")
                    for j in range(NT):
                        nc.tensor.matmul(
                            psd[:, pos * 16 + j : pos * 16 + j + 1],
                            V2[64:128, j * 128 : (j + 1) * 128],
                            rhsd[64:128, :],
                            start=True,
                            stop=True,
                        )
                    if pos == 31 or ti == t_steps - 1:
                        nc.vector.tensor_copy(
                            out_sb[:, blk * 512 : blk * 512 + (pos + 1) * 16],
                            psd[:, 0 : (pos + 1) * 16],
                        )
                if t == t_steps:
                    break
                if t + 1 < t_steps:
                    load_v2(V2n, t + 1)
                for g in range(4):
                    psm = ps2.tile([128, 1024], F32, tag="psm")
                    for jj in range(4):
                        j = 4 * g + jj
                        nc.tensor.matmul(
                            psm[:, jj * 256 : (jj + 1) * 256],
                            V2[0:128, j * 128 : (j + 1) * 128],
                            rhsm[:, :],
                            start=True,
                            stop=True,
                        )
                    sigm = work.tile([128, 768], BF, tag="sigm")
                    tgm = work.tile([128, 256], BF, tag="tgm")
                    tcm = work.tile([128, 256], BF, tag="tcm")
                    t1m = work.tile([128, 256], BF, tag="t1m")
                    t2m = work.tile([128, 256], BF, tag="t2m")
                    hm = work.tile([128, 256], BF, tag="hm")
                    psv = psm[:, :].rearrange("p (j c) -> p j c", c=256)
                    sigmv = sigm[:, :].rearrange("p (j c) -> p j c", c=192)
                    tgmv = tgm[:, :].rearrange("p (j c) -> p j c", c=64)
                    tcmv = tcm[:, :].rearrange("p (j c) -> p j c", c=64)
                    t1mv = t1m[:, :].rearrange("p (j c) -> p j c", c=64)
                    t2mv = t2m[:, :].rearrange("p (j c) -> p j c", c=64)
                    hmv = hm[:, :].rearrange("p (j c) -> p j c", c=64)
                    cg = c2[:, g * 256 : (g + 1) * 256].rearrange(
                        "p (j c) -> p j c", c=64
                    )
                    nc.scalar.activation(sigmv[:, :, :], psv[:, :, 0:192], AF.Sigmoid)
                    nc.scalar.activation(tgmv[:, :, :], psv[:, :, 192:256], AF.Tanh)
                    nc.vector.tensor_mul(
                        t1mv[:, :, :], sigmv[:, :, 64:128], cg[:, :, :]
                    )
                    nc.vector.tensor_mul(
                        t2mv[:, :, :], sigmv[:, :, 0:64], tgmv[:, :, :]
                    )
                    nc.vector.tensor_add(cg[:, :, :], t1mv[:, :, :], t2mv[:, :, :])
                    nc.scalar.activation(tcmv[:, :, :], cg[:, :, :], AF.Tanh)
                    nc.vector.tensor_mul(
                        hmv[:, :, :], sigmv[:, :, 128:192], tcmv[:, :, :]
                    )
                    pstm = ps2t.tile([64, 512], BF, tag="pstm")
                    for q in range(4):
                        nc.tensor.transpose(
                            pstm[0:64, q * 128 : (q + 1) * 128],
                            hm[:, q * 64 : (q + 1) * 64],
                            ident[:, :],
                        )
                    nc.vector.tensor_copy(
                        V2n[64:128, g * 512 : (g + 1) * 512], pstm[0:64, :]
                    )

            nc.sync.dma_start(out=out_d[:, :], in_=out_sb[:, :])

    nc.finalize()
    return nc


def prepare_inputs(inputs, t_steps=T):
    f32 = np.float32
    x = np.asarray(inputs["x"], dtype=f32)[:t_steps]  # [T, B, 8]

    p1 = _perm4(H1)
    p2 = _perm4(H2)

    Whh_f = np.asarray(inputs["Whh_f"], f32)[p1]
    Whh_b = np.asarray(inputs["Whh_b"], f32)[p1]
    Wih_f = np.asarray(inputs["Wih_f"], f32)[p1]
    Wih_b = np.asarray(inputs["Wih_b"], f32)[p1]
    b_f = (np.asarray(inputs["bih_f"], f32) + np.asarray(inputs["bhh_f"], f32))[p1]
    b_b = (np.asarray(inputs["bih_b"], f32) + np.asarray(inputs["bhh_b"], f32))[p1]
    blk = np.zeros((64, 128), f32)
    blk[0:16, 0:64] = Whh_f.T
    blk[16:32, 64:128] = Whh_b.T
    blk[32:40, 0:64] = Wih_f.T
    blk[40, 0:64] = b_f
    blk[41:49, 64:128] = Wih_b.T
    blk[49, 64:128] = b_b
    rhs50 = np.tile(blk, (2, 1)).astype(BF16NP)  # [128, 128], rows 0/64

    Wih_m = np.asarray(inputs["Wih_m"], f32)[p2]  # [256, 32]
    Whh_m = np.asarray(inputs["Whh_m"], f32)[p2]  # [256, 64]
    b_m = (np.asarray(inputs["bih_m"], f32) + np.asarray(inputs["bhh_m"], f32))[p2]
    Wd = np.asarray(inputs["Wd"], f32)[0]  # [64]
    rhs_m = np.zeros((128, 256), f32)
    rhs_m[0:16] = Wih_m[:, 0:16].T
    rhs_m[16:32] = Wih_m[:, 16:32].T
    rhs_m[32] = b_m
    rhs_m[64:128] = Whh_m.T
    rhs_m = rhs_m.astype(BF16NP)
    rhs_d = Wd.reshape(64, 1).astype(BF16NP)

    ident = np.eye(128, dtype=BF16NP)
    onesrow = np.ones((1, BS), BF16NP)

    h0f = np.asarray(inputs["h0f"], f32)
    h0b = np.asarray(inputs["h0b"], f32)
    c0f = np.asarray(inputs["c0f"], f32)
    c0b = np.asarray(inputs["c0b"], f32)
    h0m = np.asarray(inputs["h0m"], f32)
    c0m = np.asarray(inputs["c0m"], f32)

    in_maps = []
    for c in range(NCORES):
        bs = c * BS
        xc = x[:, bs : bs + BS, :]  # [T, 2048, 8]
        xpad = np.ones((t_steps, 9, BS), f32)
        xpad[:, 0:8, :] = xc.transpose(0, 2, 1)

        # HT2 h0: tile j at rows 64*(j%2)+{hf 0:16, hb 16:32}, col 128*(j//2)
        h0p1 = np.zeros((128, 1024), f32)
        c0p1 = np.zeros((128, 512), f32)
        c0mS = np.zeros((128, 1024), f32)
        for j in range(NT):
            rows = bs + j * 128
            P, cb = 64 * (j % 2), j // 2
            h0p1[P : P + 16, cb * 128 : (cb + 1) * 128] = h0f[rows : rows + 128].T
            h0p1[P + 16 : P + 32, cb * 128 : (cb + 1) * 128] = h0b[
                rows : rows + 128
            ].T
            c0p1[:, 32 * j : 32 * j + 16] = c0f[rows : rows + 128]
            c0p1[:, 32 * j + 16 : 32 * j + 32] = c0b[rows : rows + 128]
            c0mS[:, j * 64 : (j + 1) * 64] = c0m[rows : rows + 128]

        in_maps.append(
            {
                "xpad": xpad.astype(BF16NP),
                "rhs50": rhs50,
                "rhs_m": rhs_m,
                "rhs_d": rhs_d,
                "h0p1": h0p1.astype(BF16NP),
                "h0mT": h0m[bs : bs + BS].T.astype(BF16NP),
                "c0p1": c0p1.astype(BF16NP),
                "c0m": c0mS.astype(BF16NP),
                "ident": ident,
                "onesrow": onesrow,
            }
        )
    return in_maps


def unshard_output(results, bd, t_steps=T):
    outs = []
    for c in range(NCORES):
        oc = np.asarray(results[c]["out"], dtype=np.float32)  # [128, T*16]
        oc = oc.reshape(128, t_steps, NT).transpose(2, 0, 1).reshape(BS, t_steps)
        outs.append(oc)
    full = np.concatenate(outs, axis=0)  # [B, T]
    full += bd
    return full


_CACHED = {}


def kernel(**inputs):
    from concourse.bass_utils import run_bass_kernel_spmd

    t_steps = T
    if "prog" not in _CACHED:
        _CACHED["prog"] = build_program(t_steps)
    nc = _CACHED["prog"]
    in_maps = prepare_inputs(inputs, t_steps)
    res = run_bass_kernel_spmd(nc, in_maps, list(range(NCORES)))
    bd = float(np.asarray(inputs["bd"], np.float32)[0])
    return unshard_output(res.results, bd, t_steps)


if __name__ == "__main__":
    import reference

    inputs = reference.setup_inputs()
    out = kernel(**{k: np.asarray(v) for k, v in inputs.items()})
    print("kernel out", out.shape, out.dtype)
